# revision 1
# baseline (speedup 1.0000x reference)
"""KernelPoolingLayer (KNRM-style Gaussian kernel pooling) on 8 trn2 cores.

Math per output [l, b, k]:
  out = sum_q oov[b,q] * 0.01 * log(clip(sum_d m[b,q,d]*exp(-(x[l,b,q,d]-mu_k)^2/(2 s_k^2)), 1e-10))
  mu = [1.0, 0.9, 0.7, ..., -0.9]  (K=11), sigma = [0.001, 0.1, ..., 0.1]

Strategy (per core, B sharded 8 ways -> Bc=8, rows = L*Bc*Q = 1024, D=1024):
  - For sigma=0.1 kernels the Gaussians form a geometric chain:
      E_{k+1} = E_k * R * e^{-4(k-1)},  R = exp(-20x+16)
    so with a per-step rescale beta=e^-4 the stored chain tensor
      G_k = E_k * phi_k,  phi_k = e^{2(k-1)(k-4)}
    stays in fp32 range for x in [0,1]. One fused DVE tensor_tensor_reduce per
    kernel produces BOTH the next chain tensor and its D-sum.
  - Anchors (k=1), the narrow k=0 kernel, and (fast path) k=9, k=10 are computed
    on ACT (Square + Exp with fused accum_out reduction).
  - clip/log/oov and the final sum over q happen on a tiny [128, 88] stats tile;
    the q-sum (a partition-axis sum over 64 rows) is one tiny PE matmul per tile
    against a block-ones matrix.
"""

import numpy as np

L, B, Q, D = 2, 64, 64, 1024
NCORES = 8
Bc = B // NCORES            # 8
ROWS = L * Bc * Q           # 1024 rows per core
P = 128                     # partitions
NT = ROWS // P              # 8 tiles per core
K = 11
SC = NT * K                 # 88 stats columns
AUXC = 2                    # ones2 block matrix for the q-sum matmul

MU = [1.0] + [0.9 - 0.2 * (k - 1) for k in range(1, K)]


def _phis(chain_ks):
    """phi_k = e^{2(k-1)(k-4)} for chain kernels, 1 for direct kernels."""
    lnphi = np.zeros(K, np.float64)
    for k in chain_ks:
        lnphi[k] = 2.0 * (k - 1) * (k - 4)
    return lnphi


def _build_aux():
    aux = np.zeros((P, AUXC), np.float32)
    aux[:64, 0] = 1.0  # ones2 col 0: rows-group 0 of each tile
    aux[64:, 1] = 1.0  # ones2 col 1: rows-group 1
    return aux


_CACHE = {}
LAST_RESULT = None
TRACE = False


def _get_built(fast):
    if fast in _CACHE:
        return _CACHE[fast]

    from contextlib import ExitStack
    import concourse.bacc as bacc
    import concourse.mybir as mybir
    import concourse.tile as tile

    f32 = mybir.dt.float32
    AF = mybir.ActivationFunctionType
    OP = mybir.AluOpType
    E4 = float(np.exp(-4.0))

    # fast path: masks are all ones -> ACT accum_out does the k=0,1,9,10 sums
    # general path: apply query_by_doc_mask via DVE (chain propagates the mask)
    direct_ks = (9, 10) if fast else ()
    chain_ks = tuple(k for k in range(2, K) if k not in direct_ks)

    nc = bacc.Bacc(
        "TRN2", target_bir_lowering=False, debug=False, num_devices=NCORES
    )
    x_d = nc.dram_tensor("x", [ROWS, D], f32, kind="ExternalInput").ap()
    ov_d = nc.dram_tensor("ov", [P, SC], f32, kind="ExternalInput").ap()
    aux_d = nc.dram_tensor("aux", [P, AUXC], f32, kind="ExternalInput").ap()
    if not fast:
        m_d = nc.dram_tensor("m", [Bc * Q, D], f32, kind="ExternalInput").ap()
    o_d = nc.dram_tensor("o", [K, 2 * NT], f32, kind="ExternalOutput").ap()

    with tile.TileContext(nc) as tc, ExitStack() as ctx:
        xin = ctx.enter_context(tc.tile_pool(name="xin", bufs=3))
        wk = ctx.enter_context(tc.tile_pool(name="wk", bufs=2))
        gp = ctx.enter_context(tc.tile_pool(name="gp", bufs=3))
        singles = ctx.enter_context(tc.tile_pool(name="singles", bufs=1))
        psum = ctx.enter_context(tc.tile_pool(name="psum", bufs=1, space="PSUM"))

        auxt = singles.tile([P, AUXC], f32)
        nc.sync.dma_start(out=auxt, in_=aux_d)
        ovt = singles.tile([P, SC], f32)
        nc.sync.dma_start(out=ovt, in_=ov_d)
        S = singles.tile([P, SC], f32)
        if not fast:
            mts = []
            for j in range(Bc * Q // P):  # 4 mask tiles, reused for l=0/1
                mt = singles.tile([P, D], f32, tag=f"m{j}")
                nc.sync.dma_start(out=mt, in_=m_d[j * P:(j + 1) * P, :])
                mts.append(mt)

        ONES2 = auxt[:, 0:2]

        consts = {}

        def c_ap(v):
            v = float(v)
            if v not in consts:
                t = singles.tile([P, 1], f32, tag=f"cst{len(consts)}")
                nc.vector.memset(t, v)
                consts[v] = t
            return consts[v]

        for t in range(NT):
            xt = xin.tile([P, D], f32, tag="x")
            nc.sync.dma_start(out=xt, in_=x_d[t * P:(t + 1) * P, :])
            col = lambda k: S[:, t * K + k:t * K + k + 1]

            # --- anchors on ACT ---
            sq = wk.tile([P, D], f32, tag="sq")
            nc.scalar.activation(sq, xt, AF.Square, bias=c_ap(-MU[1]))
            E1 = wk.tile([P, D], f32, tag="e1")
            if fast:
                nc.scalar.activation(E1, sq, AF.Exp, scale=c_ap(-50.0),
                                     accum_out=col(1))
            else:
                nc.scalar.activation(E1, sq, AF.Exp, scale=c_ap(-50.0))
            R = wk.tile([P, D], f32, tag="r")
            nc.scalar.activation(R, xt, AF.Exp, scale=c_ap(-20.0), bias=c_ap(16.0))

            # --- k=0: narrow sigma=0.001 kernel ---
            sq0 = wk.tile([P, D], f32, tag="sq0")
            nc.scalar.activation(sq0, xt, AF.Square, bias=c_ap(-MU[0]))
            E0 = wk.tile([P, D], f32, tag="e0")
            if fast:
                nc.scalar.activation(E0, sq0, AF.Exp, scale=c_ap(-500000.0),
                                     accum_out=col(0))
            else:
                nc.scalar.activation(E0, sq0, AF.Exp, scale=c_ap(-500000.0))

            # --- fast path: k=9,10 directly on ACT; general: via chain ---
            for k in direct_ks:
                sqk = wk.tile([P, D], f32, tag="sq")
                nc.scalar.activation(sqk, xt, AF.Square, bias=c_ap(-MU[k]))
                Ek = wk.tile([P, D], f32, tag="e0")
                nc.scalar.activation(Ek, sqk, AF.Exp, scale=c_ap(-50.0),
                                     accum_out=col(k))

            # --- mask application (general path) ---
            if not fast:
                mt = mts[t % len(mts)]
                E1m = gp.tile([P, D], f32, tag="g")
                nc.vector.scalar_tensor_tensor(
                    out=E1m, in0=E1, scalar=1.0, in1=mt,
                    op0=OP.mult, op1=OP.mult, accum_out=col(1))
                E0m = wk.tile([P, D], f32, tag="e0m")
                nc.vector.scalar_tensor_tensor(
                    out=E0m, in0=E0, scalar=1.0, in1=mt,
                    op0=OP.mult, op1=OP.mult, accum_out=col(0))
                G = E1m
            else:
                G = E1

            # --- geometric chain on DVE: one fused mult+reduce per kernel ---
            for k in chain_ks:
                Gn = gp.tile([P, D], f32, tag="g")
                nc.vector.scalar_tensor_tensor(
                    out=Gn, in0=G, scalar=float(np.exp(-4.0 * (k - 2))),
                    in1=R, op0=OP.mult, op1=OP.mult, accum_out=col(k))
                G = Gn

        # --- tiny stage: clip/log/oov then q-sum via PE ---
        U = singles.tile([P, SC], f32)
        nc.vector.tensor_scalar_max(U, S, 1e-10)
        LG = singles.tile([P, SC], f32)
        nc.scalar.activation(LG, U, AF.Ln)
        V = singles.tile([P, SC], f32)
        nc.vector.tensor_mul(V, LG, ovt)

        ps = psum.tile([P, 2 * NT], f32)
        for t in range(NT):
            nc.tensor.matmul(
                out=ps[0:K, 2 * t:2 * t + 2],
                lhsT=V[:, t * K:(t + 1) * K], rhs=ONES2,
                start=True, stop=True)
        OT = singles.tile([P, 2 * NT], f32)
        nc.vector.tensor_copy(OT[0:K, :], ps[0:K, :])
        nc.sync.dma_start(out=o_d, in_=OT[0:K, :])

    nc.compile()
    _CACHE[fast] = nc
    return nc


def kernel(match_matrices, query_by_doc_mask, query_pad_oov_mask):
    global LAST_RESULT
    from concourse.bass_utils import run_bass_kernel_spmd

    x = np.ascontiguousarray(np.asarray(match_matrices, dtype=np.float32))
    m = np.ascontiguousarray(np.asarray(query_by_doc_mask, dtype=np.float32))
    ov = np.ascontiguousarray(np.asarray(query_pad_oov_mask, dtype=np.float32))
    fast = bool((m == 1.0).all())

    nc = _get_built(fast)
    aux = _build_aux()

    rowsel = (np.arange(P)[:, None] + P * np.arange(NT)[None, :]) % (Bc * Q)

    in_maps = []
    for c in range(NCORES):
        xs = x[:, c * Bc:(c + 1) * Bc].reshape(ROWS, D)
        ovs = ov[c * Bc:(c + 1) * Bc].reshape(Bc * Q)
        OV = np.repeat((0.01 * ovs[rowsel]).astype(np.float32), K, axis=1)
        im = {"x": xs, "ov": np.ascontiguousarray(OV),
              "aux": aux}
        if not fast:
            im["m"] = np.ascontiguousarray(
                m[c * Bc:(c + 1) * Bc].reshape(Bc * Q, D))
        in_maps.append(im)

    LAST_RESULT = run_bass_kernel_spmd(
        nc, in_maps, core_ids=list(range(NCORES)), trace=TRACE)
    outs = [LAST_RESULT.results[c]["o"].T.reshape(L, Bc, K)
            for c in range(NCORES)]
    return np.concatenate(outs, axis=1)

